# revision 2
# baseline (speedup 1.0000x reference)
"""AttnBlock (GroupNorm -> q/k/v 1x1 conv -> single-head attention -> proj -> residual)
on 8 Trainium2 NeuronCores — fp8e4m3 DoubleRow edition.

Sharding: pure data-parallel over batch. x is [B=8, C=512, N=2048]; core b runs the
full attention block on x[b]. No collectives.

All six big matmuls (q/k/v/proj, QK^T, PV) run as fp8e4 DoubleRow (contraction of
256 = 2x128 per instruction at 0.5 cycles/row): operands live in "pair" layout
[128, 2, F] where dim-1 indexes the two 128-row k-tiles contracted together.

Precision: output = x + proj(attn), and ||proj(attn)|| ~ 0.036*||x||, so fp8
errors (~5-10% on the attention path) land at ~4e-3 overall rel err — the
residual x itself stays exact fp32.

Softmax: S^T tiles [m, n] -> exp on ScalarE (scale=1/sqrt(C), bias=-ln4 for
headroom vs fp8e4m3 max 240; the constant cancels in normalization). Denominator
via tiny fp8 DoubleRow ones-reduce matmuls accumulated in PSUM [1, 512] (zero
DVE cost), reciprocal + PE row-broadcast, folded into the PV drain.
"""

import sys

sys.path.insert(0, "/opt/trn_rl_repo")

import math
from contextlib import ExitStack

import numpy as np

import concourse.bass as bass
import concourse.bacc as bacc
import concourse.tile as tile
from concourse import mybir
from concourse.bass_utils import run_bass_kernel_spmd

P = 128
C = 512
N = 2048
B = 8
GROUPS = 32
GSZ = 16  # channels (partition rows) per group
GPT = P // GSZ  # groups per 128-channel tile = 8
CT = C // P  # 4 channel tiles
CP = CT // 2  # 2 channel-pair tiles
MT = N // P  # 16 key tiles of 128
MP = MT // 2  # 8 key-pair tiles
NBLK = N // 512  # 4 query blocks of 512
EPS = 1e-6
SCALE = float(C) ** -0.5
MLN4 = -math.log(4.0)  # exp headroom shift; cancels in softmax normalization

f32 = mybir.dt.float32
f32r = mybir.dt.float32r
bf16 = mybir.dt.bfloat16
f8 = mybir.dt.float8e4
AX = mybir.AxisListType
OP = mybir.AluOpType
AF = mybir.ActivationFunctionType
DR = mybir.MatmulPerfMode.DoubleRow


def build(reps=1):
    nc = bacc.Bacc()

    x_d = nc.declare_dram_parameter("x", [C, N], f32, False)
    gns_d = nc.declare_dram_parameter("gn_scale", [C], f32, False)
    gnb_d = nc.declare_dram_parameter("gn_bias", [C], f32, False)
    w_d = {}
    b_d = {}
    for nm in ("wq", "wk", "wv", "wp"):
        w_d[nm] = nc.declare_dram_parameter(nm, [C, C], f32, False)
    for nm in ("bq", "bk", "bv", "bp"):
        b_d[nm] = nc.declare_dram_parameter(nm, [C], f32, False)
    sel_d = nc.declare_dram_parameter("sel", [P, GPT], f32, False)
    selT_d = nc.declare_dram_parameter("selT", [GPT, P], f32, False)
    id_d = nc.declare_dram_parameter("ident", [P, P], f32, False)
    out_d = nc.declare_dram_parameter("out", [C, N], f32, True)

    with ExitStack() as ctx:
        tc = ctx.enter_context(tile.TileContext(nc))

        const = ctx.enter_context(tc.tile_pool(name="const", bufs=1))

        # x tiles first: they are the critical path (GN -> h -> projections).
        xg_pool = ctx.enter_context(tc.tile_pool(name="xg", bufs=1))
        xg = []
        for ci in range(CT):
            xt = xg_pool.tile([P, N], f32, tag=f"xg{ci}", name=f"xg{ci}")
            for hf in range(2):
                nc.sync.dma_start(
                    out=xt[:, hf * 1024 : (hf + 1) * 1024],
                    in_=x_d[ci * P : (ci + 1) * P, hf * 1024 : (hf + 1) * 1024],
                )
            xg.append(xt)

        # constants on gpsimd (SWDGE) so they do not delay the x loads
        ident = const.tile([P, P], f32, tag="ident")
        nc.gpsimd.dma_start(out=ident, in_=id_d[:, :])
        ident_b = const.tile([P, P], bf16, tag="ident_b")
        nc.vector.tensor_copy(out=ident_b, in_=ident)
        sel_sb = const.tile([P, GPT], f32, tag="sel")
        nc.gpsimd.dma_start(out=sel_sb, in_=sel_d[:, :])
        sel_r = const.tile([P, GPT], f32r, tag="sel_r")
        nc.vector.tensor_copy(out=sel_r, in_=sel_sb)
        selT_sb = const.tile([GPT, P], f32, tag="selT")
        nc.gpsimd.dma_start(out=selT_sb, in_=selT_d[:, :])
        selT_r = const.tile([GPT, P], f32r, tag="selT_r")
        nc.vector.tensor_copy(out=selT_r, in_=selT_sb)
        ones_sb = const.tile([1, P], f32, tag="ones")
        nc.vector.memset(ones_sb, 1.0)
        ones_r = const.tile([1, P], f32r, tag="ones_r")
        nc.vector.tensor_copy(out=ones_r, in_=ones_sb)
        # [p, i, j] all-ones: DoubleRow csum lhsT. s3_lw_dual_fp8_restrictions
        # requires the pair-dim step to be a multiple of 16 bytes and M>=2
        # (out row 1 is a duplicate), hence the padded [128, 2, 16] tile.
        ones8 = const.tile([P, 2, 16], f8, tag="ones8")
        nc.vector.memset(ones8, 1.0)
        mln4_sb = const.tile([P, 1], f32, tag="mln4")
        nc.vector.memset(mln4_sb, MLN4)
        eps_sb = const.tile([GPT, 1], f32, tag="eps")
        nc.vector.memset(eps_sb, EPS)

        # per-channel vectors as [128, 4] tiles in one DMA each (col-major view)
        gs_sb = const.tile([P, CT], f32, tag="gs")
        gb_sb = const.tile([P, CT], f32, tag="gb")
        bq_sb = const.tile([P, CT], f32, tag="bq")
        bk_sb = const.tile([P, CT], f32, tag="bk")
        bp_sb = const.tile([P, CT], f32, tag="bp")
        for dst, src_d in (
            (gs_sb, gns_d),
            (gb_sb, gnb_d),
            (bq_sb, b_d["bq"]),
            (bk_sb, b_d["bk"]),
            (bp_sb, b_d["bp"]),
        ):
            nc.gpsimd.dma_start(out=dst, in_=src_d[:].rearrange("(c p) -> p c", p=P))
        # bv broadcast pair tile [128, 2, 512] (both halves identical)
        bv2 = const.tile([P, 2, C], f32, tag="bv2")
        for i in range(2):
            nc.gpsimd.dma_start(
                out=bv2[:, i, :], in_=b_d["bv"][:].unsqueeze(0).to_broadcast([P, C])
            )

        for _rep in range(reps):
            _build_body(nc, tc, ctx, locals())

    nc.finalize()
    return nc


def _build_body(nc, tc, ctx, env):
    w_d = env["w_d"]
    out_d = env["out_d"]
    const = env["const"]
    ident_b = env["ident_b"]
    sel_r = env["sel_r"]
    selT_r = env["selT_r"]
    ones_r = env["ones_r"]
    ones8 = env["ones8"]
    mln4_sb = env["mln4_sb"]
    eps_sb = env["eps_sb"]
    gs_sb = env["gs_sb"]
    gb_sb = env["gb_sb"]
    bq_sb = env["bq_sb"]
    bk_sb = env["bk_sb"]
    bp_sb = env["bp_sb"]
    bv2 = env["bv2"]
    xg = env["xg"]

    body_scope = ExitStack()
    wt_p_pool = body_scope.enter_context(tc.tile_pool(name="wt_p", bufs=1))
    qk_pool = body_scope.enter_context(tc.tile_pool(name="qk", bufs=1))
    vt_pool = body_scope.enter_context(tc.tile_pool(name="vt", bufs=1))
    # [128, 1024] f32 = 2 PSUM banks per buf; shared by q/k/v projections and QK
    psum_qk = body_scope.enter_context(tc.tile_pool(name="psum_qk", bufs=2, space="PSUM"))
    qkv_scope = ExitStack()
    wt_qkv_pool = qkv_scope.enter_context(tc.tile_pool(name="wt_qkv", bufs=1))
    h_pool = qkv_scope.enter_context(tc.tile_pool(name="h", bufs=1))

    # ---- GroupNorm ----
    h8 = [h_pool.tile([P, 2, N], f8, tag=f"h8{t}", name=f"h8{t}") for t in range(CP)]
    with tc.tile_pool(name="gn_tmp", bufs=4) as gn_tmp, tc.tile_pool(
        name="psum_gn", bufs=2, space="PSUM"
    ) as psum_gn:
        stats4 = const.tile([P, 2 * CT], f32, tag="stats4")
        for ci in range(CT):
            xt = xg[ci]
            st = gn_tmp.tile([P, 4, 6], f32, tag="st")
            for j in range(4):
                nc.vector.bn_stats(out=st[:, j, :], in_=xt[:, j * 512 : (j + 1) * 512])
            mv = gn_tmp.tile([P, 2], f32, tag="mv")
            nc.vector.bn_aggr(out=mv, in_=st)
            nc.vector.tensor_copy(out=stats4[:, ci : ci + 1], in_=mv[:, 0:1])
            # E[x^2] = mean^2 + var
            nc.vector.tensor_tensor(
                out=stats4[:, CT + ci : CT + ci + 1],
                in0=mv[:, 0:1],
                in1=mv[:, 0:1],
                op=OP.mult,
            )
            nc.vector.tensor_add(
                out=stats4[:, CT + ci : CT + ci + 1],
                in0=stats4[:, CT + ci : CT + ci + 1],
                in1=mv[:, 1:2],
            )
        # group-aggregate across the 16-row groups of each tile
        stats4_r = const.tile([P, 2 * CT], f32r, tag="stats4_r")
        nc.vector.tensor_copy(out=stats4_r, in_=stats4)
        psg = psum_gn.tile([GPT, 2 * CT], f32, tag="psg")
        nc.tensor.matmul(psg, sel_r, stats4_r, start=True, stop=True)
        g2 = const.tile([GPT, 2 * CT], f32, tag="g2")
        gtmp = const.tile([GPT, 2 * CT], f32, tag="gtmp")
        nc.vector.tensor_scalar_mul(g2[:, 0:CT], psg[:, 0:CT], 1.0 / GSZ)
        nc.vector.tensor_scalar_mul(gtmp[:, 0:CT], psg[:, CT : 2 * CT], 1.0 / GSZ)
        nc.vector.tensor_tensor(
            out=gtmp[:, CT : 2 * CT], in0=g2[:, 0:CT], in1=g2[:, 0:CT], op=OP.mult
        )
        nc.vector.tensor_sub(gtmp[:, 0:CT], gtmp[:, 0:CT], gtmp[:, CT : 2 * CT])
        nc.scalar.activation(
            out=gtmp[:, 0:CT], in_=gtmp[:, 0:CT], func=AF.Sqrt, bias=eps_sb, scale=1.0
        )
        nc.vector.reciprocal(out=g2[:, CT : 2 * CT], in_=gtmp[:, 0:CT])
        # broadcast per-group stats back to the 128 rows of each tile
        g2_r = const.tile([GPT, 2 * CT], f32r, tag="g2_r")
        nc.vector.tensor_copy(out=g2_r, in_=g2)
        psb = psum_gn.tile([P, 2 * CT], f32, tag="psb")
        nc.tensor.matmul(psb, selT_r, g2_r, start=True, stop=True)
        rowst = const.tile([P, 2 * CT], f32, tag="rowst")
        nc.vector.tensor_copy(out=rowst, in_=psb)
        # fold gn scale/bias: h = A*x + B with A = rstd*scale, B = bias - mean*A
        AB = const.tile([P, 2 * CT], f32, tag="AB")
        for ci in range(CT):
            nc.vector.tensor_tensor(
                out=AB[:, ci : ci + 1],
                in0=rowst[:, CT + ci : CT + ci + 1],
                in1=gs_sb[:, ci : ci + 1],
                op=OP.mult,
            )
            nc.vector.tensor_tensor(
                out=AB[:, CT + ci : CT + ci + 1],
                in0=rowst[:, ci : ci + 1],
                in1=AB[:, ci : ci + 1],
                op=OP.mult,
            )
            nc.vector.tensor_sub(
                AB[:, CT + ci : CT + ci + 1],
                gb_sb[:, ci : ci + 1],
                AB[:, CT + ci : CT + ci + 1],
            )
        # h in fp8 pair layout; split across Scalar (2) and DVE (2)
        for ci in range(CT):
            dst = h8[ci // 2][:, ci % 2, :]
            if ci % 2 == 0:
                nc.scalar.activation(
                    out=dst,
                    in_=xg[ci],
                    func=AF.Identity,
                    bias=AB[:, CT + ci : CT + ci + 1],
                    scale=AB[:, ci : ci + 1],
                )
            else:
                nc.vector.tensor_scalar(
                    out=dst,
                    in0=xg[ci],
                    scalar1=AB[:, ci : ci + 1],
                    scalar2=AB[:, CT + ci : CT + ci + 1],
                    op0=OP.mult,
                    op1=OP.add,
                )

    # ---- weights: load [o, c] as bf16, PE-transpose into fp8 pair tiles ----
    # w8T[nm][t] is [128, 2, 512]: [p, i, o] = w[o, c=128*(2t+i)+p]
    w8T = {}
    ncopy = 0
    with tc.tile_pool(name="wraw", bufs=2) as wraw, tc.tile_pool(
        name="psum_w", bufs=2, space="PSUM"
    ) as psum_w:
        for nm in ("wq", "wk", "wv", "wp"):
            pool_w = wt_p_pool if nm == "wp" else wt_qkv_pool
            w8T[nm] = [
                pool_w.tile([P, 2, C], f8, tag=f"w8T_{nm}{t}", name=f"w8T_{nm}{t}")
                for t in range(CP)
            ]
            raws = []
            for oi in range(CT):
                raw = wraw.tile([P, C], bf16, tag="wraw", bufs=8, name="raw")
                nc.gpsimd.dma_start(out=raw, in_=w_d[nm][oi * P : (oi + 1) * P, :])
                raws.append(raw)
            for ci in range(CT):
                psT = psum_w.tile([P, C], bf16, tag="psT", name="psT")
                for oi in range(CT):
                    nc.tensor.transpose(
                        psT[:, oi * P : (oi + 1) * P],
                        raws[oi][:, ci * P : (ci + 1) * P],
                        ident_b,
                    )
                dst = w8T[nm][ci // 2][:, ci % 2, :]
                if ncopy % 2 == 0:
                    nc.vector.tensor_copy(out=dst, in_=psT)
                else:
                    nc.scalar.copy(out=dst, in_=psT)
                ncopy += 1

    # ---- q/k projections (fp8 pair layout [128, 2, 2048]) ----
    q8 = [qk_pool.tile([P, 2, N], f8, tag=f"q8{t}", name=f"q8{t}") for t in range(CP)]
    k8 = [qk_pool.tile([P, 2, N], f8, tag=f"k8{t}", name=f"k8{t}") for t in range(CP)]
    for nm, dst8, bias_sb in (("wq", q8, bq_sb), ("wk", k8, bk_sb)):
        for oi in range(CT):
            for nbp in range(2):
                ps = psum_qk.tile([P, 1024], f32, tag="ps_qk", name="ps_qk")
                for half in range(2):
                    nsl = (nbp * 2 + half) * 512
                    for t in range(CP):
                        nc.tensor.matmul(
                            ps[:, half * 512 : (half + 1) * 512],
                            w8T[nm][t][:, :, oi * P : (oi + 1) * P],
                            h8[t][:, :, nsl : nsl + 512],
                            start=(t == 0),
                            stop=(t == CP - 1),
                            perf_mode=DR,
                        )
                out_ap = dst8[oi // 2][:, oi % 2, nbp * 1024 : (nbp + 1) * 1024]
                if nm == "wq":
                    nc.scalar.activation(
                        out=out_ap,
                        in_=ps,
                        func=AF.Identity,
                        bias=bias_sb[:, oi : oi + 1],
                    )
                else:
                    nc.vector.tensor_scalar_add(out_ap, ps, bias_sb[:, oi : oi + 1])

    # ---- v projection into fp8 m-pair tiles [128, 2, 512]: [p, i, c] ----
    v8 = [
        vt_pool.tile([P, 2, C], f8, tag=f"v8{mp}", name=f"v8{mp}") for mp in range(MP)
    ]
    for mp in range(MP):
        ps = psum_qk.tile([P, 2, 512], f32, tag="ps_qk", name="ps_v")
        for i in range(2):
            mi = 2 * mp + i
            for t in range(CP):
                nc.tensor.matmul(
                    ps[:, i, :],
                    h8[t][:, :, mi * P : (mi + 1) * P],
                    w8T["wv"][t],
                    start=(t == 0),
                    stop=(t == CP - 1),
                    perf_mode=DR,
                )
        nc.vector.tensor_tensor(out=v8[mp], in0=ps, in1=bv2, op=OP.add)
    qkv_scope.close()

    # ---- attention + fused output projection ----
    with tc.tile_pool(name="ptr", bufs=2) as pt_pool, tc.tile_pool(
        name="sm", bufs=2
    ) as sm_pool, tc.tile_pool(name="h2", bufs=2) as h2_pool, tc.tile_pool(
        name="outp", bufs=4
    ) as out_pool, tc.tile_pool(
        name="psum_pv", bufs=2, space="PSUM"
    ) as psum_pv, tc.tile_pool(
        name="psum_cs", bufs=1, space="PSUM"
    ) as psum_cs, tc.tile_pool(name="psum_o", bufs=1, space="PSUM") as psum_o:
        for qb in range(NBLK):
            nq = qb * 512
            PT = [
                pt_pool.tile([P, 2, 512], f8, tag=f"pt{mp}", name="pt")
                for mp in range(MP)
            ]
            cs_ps = psum_cs.tile([2, 512], f32, tag="cs", name="cs")
            for mp in range(MP):
                ps_s = psum_qk.tile([P, 2, 512], f32, tag="ps_qk", name="ps_s")
                for i in range(2):
                    mi = 2 * mp + i
                    for t in range(CP):
                        nc.tensor.matmul(
                            ps_s[:, i, :],
                            k8[t][:, :, mi * P : (mi + 1) * P],
                            q8[t][:, :, nq : nq + 512],
                            start=(t == 0),
                            stop=(t == CP - 1),
                            perf_mode=DR,
                        )
                nc.scalar.activation(
                    out=PT[mp], in_=ps_s, func=AF.Exp, scale=SCALE, bias=mln4_sb
                )
                # denominator: ones-reduce over the m pair, accumulated in PSUM
                nc.tensor.matmul(
                    cs_ps,
                    ones8[:, :, 0:2],
                    PT[mp],
                    start=(mp == 0),
                    stop=(mp == MP - 1),
                    perf_mode=DR,
                )
            # 1/colsum broadcast across partitions
            rinv = sm_pool.tile([1, 512], f32r, tag="rinv")
            with nc.allow_low_precision(reason="f32r softmax normalizer"):
                nc.vector.reciprocal(out=rinv, in_=cs_ps[0:1, :])
            ps_R = psum_o.tile([P, 512], f32, tag="ps_o", name="ps_R")
            nc.tensor.matmul(ps_R, ones_r, rinv, start=True, stop=True)
            Rsb = sm_pool.tile([P, 512], f32, tag="Rsb")
            nc.scalar.copy(out=Rsb, in_=ps_R)
            # PV into h2 fp8 c-pair tiles [128, 2, 512], normalizer folded in
            h28 = [
                h2_pool.tile([P, 2, 512], f8, tag=f"h2{t}", name="h2")
                for t in range(CP)
            ]
            for t2 in range(CP):
                for i2 in range(2):
                    ci = 2 * t2 + i2
                    ps_pv = psum_pv.tile([P, 512], f32, tag="ps_pv", name="ps_pv")
                    for mp in range(MP):
                        nc.tensor.matmul(
                            ps_pv,
                            v8[mp][:, :, ci * P : (ci + 1) * P],
                            PT[mp],
                            start=(mp == 0),
                            stop=(mp == MP - 1),
                            perf_mode=DR,
                        )
                    nc.vector.tensor_tensor(
                        out=h28[t2][:, i2, :], in0=ps_pv, in1=Rsb, op=OP.mult
                    )
            # output projection + bias + residual
            for oi in range(CT):
                ps_o = psum_o.tile([P, 512], f32, tag="ps_o", name="ps_o")
                for t2 in range(CP):
                    nc.tensor.matmul(
                        ps_o,
                        w8T["wp"][t2][:, :, oi * P : (oi + 1) * P],
                        h28[t2],
                        start=(t2 == 0),
                        stop=(t2 == CP - 1),
                        perf_mode=DR,
                    )
                ot = out_pool.tile([P, 512], f32, tag="out")
                nc.vector.scalar_tensor_tensor(
                    out=ot,
                    in0=ps_o,
                    scalar=bp_sb[:, oi : oi + 1],
                    in1=xg[oi][:, nq : nq + 512],
                    op0=OP.add,
                    op1=OP.add,
                )
                nc.gpsimd.dma_start(
                    out=out_d[oi * P : (oi + 1) * P, nq : nq + 512],
                    in_=ot,
                )
    body_scope.close()


_NC = {}


def _get_nc(reps=1):
    if reps not in _NC:
        _NC[reps] = build(reps)
    return _NC[reps]


def _consts():
    sel = np.zeros((P, GPT), np.float32)
    for rr in range(P):
        sel[rr, rr // GSZ] = 1.0
    selT = sel.T.copy()
    ident = np.eye(P, dtype=np.float32)
    return sel, selT, ident


def make_in_maps(inputs):
    x = np.ascontiguousarray(np.asarray(inputs["x"], dtype=np.float32))
    common = {}
    for nm in ("gn_scale", "gn_bias", "wq", "bq", "wk", "bk", "wv", "bv", "wp", "bp"):
        common[nm] = np.ascontiguousarray(np.asarray(inputs[nm], dtype=np.float32))
    sel, selT, ident = _consts()
    common["sel"] = sel
    common["selT"] = selT
    common["ident"] = ident
    return [dict(common, x=x[b]) for b in range(B)]


_EXEC = {}


def _get_exec(nc):
    """Build (once) the sharded jitted callable for the 8-core SPMD program."""
    key = id(nc)
    if key in _EXEC:
        return _EXEC[key]
    import jax
    from jax.sharding import Mesh, NamedSharding, PartitionSpec
    from jax.experimental.shard_map import shard_map
    from concourse.bass2jax import _bass_exec_p, install_neuronx_cc_hook

    install_neuronx_cc_hook()
    in_names, out_names, out_avals = [], [], []
    for alloc in nc.m.functions[0].allocations:
        if not isinstance(alloc, mybir.MemoryLocationSet):
            continue
        name = alloc.memorylocations[0].name
        if alloc.kind == "ExternalInput":
            in_names.append(name)
        elif alloc.kind == "ExternalOutput":
            out_names.append(name)
            out_avals.append(
                jax.core.ShapedArray(tuple(alloc.tensor_shape), mybir.dt.np(alloc.dtype))
            )
    all_names = in_names + out_names

    def _body(*args):
        return tuple(
            _bass_exec_p.bind(
                *args,
                out_avals=tuple(out_avals),
                in_names=tuple(all_names),
                out_names=tuple(out_names),
                lowering_input_output_aliases=(),
                sim_require_finite=True,
                sim_require_nnan=True,
                nc=nc,
            )
        )

    devices = jax.devices()[:B]
    mesh = Mesh(np.asarray(devices), ("core",))
    nsh = NamedSharding(mesh, PartitionSpec("core"))
    nsh_rep = NamedSharding(mesh, PartitionSpec())
    # x and partition_id differ per core; weights/consts are replicated so they
    # transfer once and fan out terminal-side.
    sharded_names = {"x", "partition_id"}
    in_specs = tuple(
        PartitionSpec("core") if nm in sharded_names else PartitionSpec()
        for nm in in_names
    ) + (PartitionSpec("core"),) * len(out_names)
    fn = jax.jit(
        shard_map(
            _body,
            mesh=mesh,
            in_specs=in_specs,
            out_specs=(PartitionSpec("core"),) * len(out_names),
            check_rep=False,
        ),
        keep_unused=True,
    )
    st = {
        "fn": fn,
        "in_names": in_names,
        "out_names": out_names,
        "out_avals": out_avals,
        "nsh": nsh,
        "nsh_rep": nsh_rep,
        "sharded_names": sharded_names,
        "hash": None,
        "dev_args": None,
    }
    _EXEC[key] = st
    return st


def kernel(_retried=False, **inputs):
    import hashlib

    import jax

    nc = _get_nc()
    st = _get_exec(nc)
    in_maps = make_in_maps(inputs)

    hsh = hashlib.md5()
    for nm in ("x", "gn_scale", "gn_bias", "wq", "bq", "wk", "bk", "wv", "bv", "wp", "bp"):
        hsh.update(np.ascontiguousarray(np.asarray(inputs[nm], np.float32)).tobytes())
    digest = hsh.digest()
    if st["hash"] != digest or st["dev_args"] is None:
        def _cv(c, nm):
            if nm in in_maps[c]:
                return np.asarray(in_maps[c][nm])
            for alloc in nc.m.functions[0].allocations:
                if (
                    isinstance(alloc, mybir.MemoryLocationSet)
                    and alloc.memorylocations[0].name == nm
                ):
                    return np.full(
                        tuple(alloc.tensor_shape), c, mybir.dt.np(alloc.dtype)
                    )
            raise KeyError(nm)

        dev_args = []
        for nm in st["in_names"]:
            if nm in st["sharded_names"]:
                a = np.concatenate([_cv(c, nm) for c in range(B)], axis=0)
                dev_args.append(jax.device_put(a, st["nsh"]))
            else:
                dev_args.append(jax.device_put(_cv(0, nm), st["nsh_rep"]))
        for a in st["out_avals"]:
            z = np.zeros((B * a.shape[0], *a.shape[1:]), a.dtype)
            dev_args.append(jax.device_put(z, st["nsh"]))
        st["dev_args"] = dev_args
        st["hash"] = digest

    try:
        r = st["fn"](*st["dev_args"])
        jax.block_until_ready(r)
    except Exception:
        # transient device error (e.g. NRT exec-unit wedge): re-place buffers
        # and retry once after a short backoff
        import time as _time

        _time.sleep(10.0)
        if _retried:
            raise
        st["hash"] = None
        st["dev_args"] = None
        return kernel(_retried=True, **inputs)
    out = np.asarray(r[0]).reshape(B, C, N)
    return out.astype(np.float32)


# revision 3
# speedup vs baseline: 1.0142x; 1.0142x over previous
"""AttnBlock (GroupNorm -> q/k/v 1x1 conv -> single-head attention -> proj -> residual)
on 8 Trainium2 NeuronCores — fp8e4m3 DoubleRow edition.

Sharding: pure data-parallel over batch. x is [B=8, C=512, N=2048]; core b runs the
full attention block on x[b]. No collectives.

All six big matmuls (q/k/v/proj, QK^T, PV) run as fp8e4 DoubleRow (contraction of
256 = 2x128 per instruction at 0.5 cycles/row): operands live in "pair" layout
[128, 2, F] where dim-1 indexes the two 128-row k-tiles contracted together.

Precision: output = x + proj(attn), and ||proj(attn)|| ~ 0.036*||x||, so fp8
errors (~5-10% on the attention path) land at ~4e-3 overall rel err — the
residual x itself stays exact fp32.

Softmax: S^T tiles [m, n] -> exp on ScalarE (scale=1/sqrt(C), bias=-ln4 for
headroom vs fp8e4m3 max 240; the constant cancels in normalization). Denominator
via tiny fp8 DoubleRow ones-reduce matmuls accumulated in PSUM [1, 512] (zero
DVE cost), reciprocal + PE row-broadcast, folded into the PV drain.
"""

import sys

sys.path.insert(0, "/opt/trn_rl_repo")

import math
from contextlib import ExitStack

import numpy as np

import concourse.bass as bass
import concourse.bacc as bacc
import concourse.tile as tile
from concourse import mybir
from concourse.bass_utils import run_bass_kernel_spmd

P = 128
C = 512
N = 2048
B = 8
GROUPS = 32
GSZ = 16  # channels (partition rows) per group
GPT = P // GSZ  # groups per 128-channel tile = 8
CT = C // P  # 4 channel tiles
CP = CT // 2  # 2 channel-pair tiles
MT = N // P  # 16 key tiles of 128
MP = MT // 2  # 8 key-pair tiles
NBLK = N // 512  # 4 query blocks of 512
EPS = 1e-6
SCALE = float(C) ** -0.5
MLN4 = -math.log(4.0)  # exp headroom shift; cancels in softmax normalization

f32 = mybir.dt.float32
f32r = mybir.dt.float32r
bf16 = mybir.dt.bfloat16
f8 = mybir.dt.float8e4
AX = mybir.AxisListType
OP = mybir.AluOpType
AF = mybir.ActivationFunctionType
DR = mybir.MatmulPerfMode.DoubleRow


def build(reps=1):
    nc = bacc.Bacc()

    x_d = nc.declare_dram_parameter("x", [C, N], f32, False)
    gns_d = nc.declare_dram_parameter("gn_scale", [C], f32, False)
    gnb_d = nc.declare_dram_parameter("gn_bias", [C], f32, False)
    w_d = {}
    b_d = {}
    for nm in ("wq", "wk", "wv", "wp"):
        w_d[nm] = nc.declare_dram_parameter(nm, [C, C], f32, False)
    for nm in ("bq", "bk", "bv", "bp"):
        b_d[nm] = nc.declare_dram_parameter(nm, [C], f32, False)
    sel_d = nc.declare_dram_parameter("sel", [P, GPT], f32, False)
    selT_d = nc.declare_dram_parameter("selT", [GPT, P], f32, False)
    id_d = nc.declare_dram_parameter("ident", [P, P], f32, False)
    out_d = nc.declare_dram_parameter("out", [C, N], f32, True)

    with ExitStack() as ctx:
        tc = ctx.enter_context(tile.TileContext(nc))

        const = ctx.enter_context(tc.tile_pool(name="const", bufs=1))

        # x tiles first: they are the critical path (GN -> h -> projections).
        xg_pool = ctx.enter_context(tc.tile_pool(name="xg", bufs=1))
        xg = []
        for ci in range(CT):
            xt = xg_pool.tile([P, N], f32, tag=f"xg{ci}", name=f"xg{ci}")
            for hf in range(2):
                nc.sync.dma_start(
                    out=xt[:, hf * 1024 : (hf + 1) * 1024],
                    in_=x_d[ci * P : (ci + 1) * P, hf * 1024 : (hf + 1) * 1024],
                )
            xg.append(xt)

        # constants on gpsimd (SWDGE) so they do not delay the x loads
        ident = const.tile([P, P], f32, tag="ident")
        nc.gpsimd.dma_start(out=ident, in_=id_d[:, :])
        ident_b = const.tile([P, P], bf16, tag="ident_b")
        nc.vector.tensor_copy(out=ident_b, in_=ident)
        sel_sb = const.tile([P, GPT], f32, tag="sel")
        nc.gpsimd.dma_start(out=sel_sb, in_=sel_d[:, :])
        sel_r = const.tile([P, GPT], f32r, tag="sel_r")
        nc.vector.tensor_copy(out=sel_r, in_=sel_sb)
        selT_sb = const.tile([GPT, P], f32, tag="selT")
        nc.gpsimd.dma_start(out=selT_sb, in_=selT_d[:, :])
        selT_r = const.tile([GPT, P], f32r, tag="selT_r")
        nc.vector.tensor_copy(out=selT_r, in_=selT_sb)
        ones_sb = const.tile([1, P], f32, tag="ones")
        nc.vector.memset(ones_sb, 1.0)
        ones_r = const.tile([1, P], f32r, tag="ones_r")
        nc.vector.tensor_copy(out=ones_r, in_=ones_sb)
        # [p, i, j] all-ones: DoubleRow csum lhsT. s3_lw_dual_fp8_restrictions
        # requires the pair-dim step to be a multiple of 16 bytes and M>=2
        # (out row 1 is a duplicate), hence the padded [128, 2, 16] tile.
        ones8 = const.tile([P, 2, 16], f8, tag="ones8")
        nc.vector.memset(ones8, 1.0)
        mln4_sb = const.tile([P, 1], f32, tag="mln4")
        nc.vector.memset(mln4_sb, MLN4)
        eps_sb = const.tile([GPT, 1], f32, tag="eps")
        nc.vector.memset(eps_sb, EPS)

        # per-channel vectors as [128, 4] tiles in one DMA each (col-major view)
        gs_sb = const.tile([P, CT], f32, tag="gs")
        gb_sb = const.tile([P, CT], f32, tag="gb")
        # bk is not loaded: a per-query logit shift cancels exactly in softmax.
        # bv is not loaded: it is folded host-side into bp (bp' = bp + wp@bv),
        # since softmax weights sum to 1.
        bq_sb = const.tile([P, CT], f32, tag="bq")
        bp_sb = const.tile([P, CT], f32, tag="bp")
        for dst, src_d in (
            (gs_sb, gns_d),
            (gb_sb, gnb_d),
            (bq_sb, b_d["bq"]),
            (bp_sb, b_d["bp"]),
        ):
            nc.gpsimd.dma_start(out=dst, in_=src_d[:].rearrange("(c p) -> p c", p=P))

        for _rep in range(reps):
            _build_body(nc, tc, ctx, locals())

    nc.finalize()
    return nc


def _build_body(nc, tc, ctx, env):
    w_d = env["w_d"]
    out_d = env["out_d"]
    const = env["const"]
    ident_b = env["ident_b"]
    sel_r = env["sel_r"]
    selT_r = env["selT_r"]
    ones_r = env["ones_r"]
    ones8 = env["ones8"]
    mln4_sb = env["mln4_sb"]
    eps_sb = env["eps_sb"]
    gs_sb = env["gs_sb"]
    gb_sb = env["gb_sb"]
    bq_sb = env["bq_sb"]
    bp_sb = env["bp_sb"]
    xg = env["xg"]

    body_scope = ExitStack()
    wt_p_pool = body_scope.enter_context(tc.tile_pool(name="wt_p", bufs=1))
    qk_pool = body_scope.enter_context(tc.tile_pool(name="qk", bufs=1))
    vt_pool = body_scope.enter_context(tc.tile_pool(name="vt", bufs=1))
    # [128, 1024] f32 = 2 PSUM banks per buf; shared by q/k/v projections and QK
    psum_qk = body_scope.enter_context(tc.tile_pool(name="psum_qk", bufs=2, space="PSUM"))
    qkv_scope = ExitStack()
    wt_qkv_pool = qkv_scope.enter_context(tc.tile_pool(name="wt_qkv", bufs=1))
    h_pool = qkv_scope.enter_context(tc.tile_pool(name="h", bufs=1))

    # ---- GroupNorm ----
    h8 = [h_pool.tile([P, 2, N], f8, tag=f"h8{t}", name=f"h8{t}") for t in range(CP)]
    with tc.tile_pool(name="gn_tmp", bufs=4) as gn_tmp, tc.tile_pool(
        name="psum_gn", bufs=2, space="PSUM"
    ) as psum_gn:
        stats4 = const.tile([P, 2 * CT], f32, tag="stats4")
        for ci in range(CT):
            xt = xg[ci]
            st = gn_tmp.tile([P, 4, 6], f32, tag="st")
            for j in range(4):
                nc.vector.bn_stats(out=st[:, j, :], in_=xt[:, j * 512 : (j + 1) * 512])
            mv = gn_tmp.tile([P, 2], f32, tag="mv")
            nc.vector.bn_aggr(out=mv, in_=st)
            nc.vector.tensor_copy(out=stats4[:, ci : ci + 1], in_=mv[:, 0:1])
            # E[x^2] = mean^2 + var
            nc.vector.tensor_tensor(
                out=stats4[:, CT + ci : CT + ci + 1],
                in0=mv[:, 0:1],
                in1=mv[:, 0:1],
                op=OP.mult,
            )
            nc.vector.tensor_add(
                out=stats4[:, CT + ci : CT + ci + 1],
                in0=stats4[:, CT + ci : CT + ci + 1],
                in1=mv[:, 1:2],
            )
        # group-aggregate across the 16-row groups of each tile
        stats4_r = const.tile([P, 2 * CT], f32r, tag="stats4_r")
        nc.vector.tensor_copy(out=stats4_r, in_=stats4)
        psg = psum_gn.tile([GPT, 2 * CT], f32, tag="psg")
        nc.tensor.matmul(psg, sel_r, stats4_r, start=True, stop=True)
        g2 = const.tile([GPT, 2 * CT], f32, tag="g2")
        gtmp = const.tile([GPT, 2 * CT], f32, tag="gtmp")
        nc.vector.tensor_scalar_mul(g2[:, 0:CT], psg[:, 0:CT], 1.0 / GSZ)
        nc.vector.tensor_scalar_mul(gtmp[:, 0:CT], psg[:, CT : 2 * CT], 1.0 / GSZ)
        nc.vector.tensor_tensor(
            out=gtmp[:, CT : 2 * CT], in0=g2[:, 0:CT], in1=g2[:, 0:CT], op=OP.mult
        )
        nc.vector.tensor_sub(gtmp[:, 0:CT], gtmp[:, 0:CT], gtmp[:, CT : 2 * CT])
        nc.scalar.activation(
            out=gtmp[:, 0:CT], in_=gtmp[:, 0:CT], func=AF.Sqrt, bias=eps_sb, scale=1.0
        )
        nc.vector.reciprocal(out=g2[:, CT : 2 * CT], in_=gtmp[:, 0:CT])
        # broadcast per-group stats back to the 128 rows of each tile
        g2_r = const.tile([GPT, 2 * CT], f32r, tag="g2_r")
        nc.vector.tensor_copy(out=g2_r, in_=g2)
        psb = psum_gn.tile([P, 2 * CT], f32, tag="psb")
        nc.tensor.matmul(psb, selT_r, g2_r, start=True, stop=True)
        rowst = const.tile([P, 2 * CT], f32, tag="rowst")
        nc.vector.tensor_copy(out=rowst, in_=psb)
        # fold gn scale/bias: h = A*x + B with A = rstd*scale, B = bias - mean*A
        AB = const.tile([P, 2 * CT], f32, tag="AB")
        for ci in range(CT):
            nc.vector.tensor_tensor(
                out=AB[:, ci : ci + 1],
                in0=rowst[:, CT + ci : CT + ci + 1],
                in1=gs_sb[:, ci : ci + 1],
                op=OP.mult,
            )
            nc.vector.tensor_tensor(
                out=AB[:, CT + ci : CT + ci + 1],
                in0=rowst[:, ci : ci + 1],
                in1=AB[:, ci : ci + 1],
                op=OP.mult,
            )
            nc.vector.tensor_sub(
                AB[:, CT + ci : CT + ci + 1],
                gb_sb[:, ci : ci + 1],
                AB[:, CT + ci : CT + ci + 1],
            )
        # h in fp8 pair layout; split across Scalar (2) and DVE (2)
        for ci in range(CT):
            dst = h8[ci // 2][:, ci % 2, :]
            if ci % 2 == 0:
                nc.scalar.activation(
                    out=dst,
                    in_=xg[ci],
                    func=AF.Identity,
                    bias=AB[:, CT + ci : CT + ci + 1],
                    scale=AB[:, ci : ci + 1],
                )
            else:
                nc.vector.tensor_scalar(
                    out=dst,
                    in0=xg[ci],
                    scalar1=AB[:, ci : ci + 1],
                    scalar2=AB[:, CT + ci : CT + ci + 1],
                    op0=OP.mult,
                    op1=OP.add,
                )

    # ---- weights: load [o, c] as bf16, PE-transpose into fp8 pair tiles ----
    # w8T[nm][t] is [128, 2, 512]: [p, i, o] = w[o, c=128*(2t+i)+p]
    w8T = {}
    ncopy = 0
    with tc.tile_pool(name="wraw", bufs=2) as wraw, tc.tile_pool(
        name="psum_w", bufs=2, space="PSUM"
    ) as psum_w:
        for nm in ("wq", "wk", "wv", "wp"):
            pool_w = wt_p_pool if nm == "wp" else wt_qkv_pool
            w8T[nm] = [
                pool_w.tile([P, 2, C], f8, tag=f"w8T_{nm}{t}", name=f"w8T_{nm}{t}")
                for t in range(CP)
            ]
            raws = []
            for oi in range(CT):
                raw = wraw.tile([P, C], bf16, tag="wraw", bufs=8, name="raw")
                nc.gpsimd.dma_start(out=raw, in_=w_d[nm][oi * P : (oi + 1) * P, :])
                raws.append(raw)
            for ci in range(CT):
                psT = psum_w.tile([P, C], bf16, tag="psT", name="psT")
                for oi in range(CT):
                    nc.tensor.transpose(
                        psT[:, oi * P : (oi + 1) * P],
                        raws[oi][:, ci * P : (ci + 1) * P],
                        ident_b,
                    )
                dst = w8T[nm][ci // 2][:, ci % 2, :]
                if ncopy % 2 == 0:
                    nc.vector.tensor_copy(out=dst, in_=psT)
                else:
                    nc.scalar.copy(out=dst, in_=psT)
                ncopy += 1

    # ---- q/k projections (fp8 pair layout [128, 2, 2048]) ----
    q8 = [qk_pool.tile([P, 2, N], f8, tag=f"q8{t}", name=f"q8{t}") for t in range(CP)]
    k8 = [qk_pool.tile([P, 2, N], f8, tag=f"k8{t}", name=f"k8{t}") for t in range(CP)]
    for nm, dst8 in (("wq", q8), ("wk", k8)):
        for oi in range(CT):
            for nbp in range(2):
                ps = psum_qk.tile([P, 1024], f32, tag="ps_qk", name="ps_qk")
                for half in range(2):
                    nsl = (nbp * 2 + half) * 512
                    for t in range(CP):
                        nc.tensor.matmul(
                            ps[:, half * 512 : (half + 1) * 512],
                            w8T[nm][t][:, :, oi * P : (oi + 1) * P],
                            h8[t][:, :, nsl : nsl + 512],
                            start=(t == 0),
                            stop=(t == CP - 1),
                            perf_mode=DR,
                        )
                out_ap = dst8[oi // 2][:, oi % 2, nbp * 1024 : (nbp + 1) * 1024]
                if nm == "wq":
                    nc.scalar.activation(
                        out=out_ap,
                        in_=ps,
                        func=AF.Identity,
                        bias=bq_sb[:, oi : oi + 1],
                    )
                else:
                    # k bias dropped (cancels in softmax) -> pure copy,
                    # split across DVE/Scalar to balance the phase
                    if (oi + nbp) % 2 == 0:
                        nc.vector.tensor_copy(out=out_ap, in_=ps)
                    else:
                        nc.scalar.copy(out=out_ap, in_=ps)

    # ---- v projection into fp8 m-pair tiles [128, 2, 512]: [p, i, c] ----
    v8 = [
        vt_pool.tile([P, 2, C], f8, tag=f"v8{mp}", name=f"v8{mp}") for mp in range(MP)
    ]
    for mp in range(MP):
        ps = psum_qk.tile([P, 2, 512], f32, tag="ps_qk", name="ps_v")
        for i in range(2):
            mi = 2 * mp + i
            for t in range(CP):
                nc.tensor.matmul(
                    ps[:, i, :],
                    h8[t][:, :, mi * P : (mi + 1) * P],
                    w8T["wv"][t],
                    start=(t == 0),
                    stop=(t == CP - 1),
                    perf_mode=DR,
                )
        # bv folded into bp host-side -> pure copy, split DVE/Scalar
        if mp % 2 == 0:
            nc.vector.tensor_copy(out=v8[mp], in_=ps)
        else:
            nc.scalar.copy(out=v8[mp], in_=ps)
    qkv_scope.close()

    # ---- attention + fused output projection ----
    with tc.tile_pool(name="ptr", bufs=2) as pt_pool, tc.tile_pool(
        name="sm", bufs=2
    ) as sm_pool, tc.tile_pool(name="h2", bufs=2) as h2_pool, tc.tile_pool(
        name="outp", bufs=4
    ) as out_pool, tc.tile_pool(
        name="psum_pv", bufs=2, space="PSUM"
    ) as psum_pv, tc.tile_pool(
        name="psum_cs", bufs=1, space="PSUM"
    ) as psum_cs, tc.tile_pool(name="psum_o", bufs=1, space="PSUM") as psum_o:
        for qb in range(NBLK):
            nq = qb * 512
            PT = [
                pt_pool.tile([P, 2, 512], f8, tag=f"pt{mp}", name="pt")
                for mp in range(MP)
            ]
            cs_ps = psum_cs.tile([2, 512], f32, tag="cs", name="cs")
            for mp in range(MP):
                ps_s = psum_qk.tile([P, 2, 512], f32, tag="ps_qk", name="ps_s")
                for i in range(2):
                    mi = 2 * mp + i
                    for t in range(CP):
                        nc.tensor.matmul(
                            ps_s[:, i, :],
                            k8[t][:, :, mi * P : (mi + 1) * P],
                            q8[t][:, :, nq : nq + 512],
                            start=(t == 0),
                            stop=(t == CP - 1),
                            perf_mode=DR,
                        )
                nc.scalar.activation(
                    out=PT[mp], in_=ps_s, func=AF.Exp, scale=SCALE, bias=mln4_sb
                )
                # denominator: ones-reduce over the m pair, accumulated in PSUM
                nc.tensor.matmul(
                    cs_ps,
                    ones8[:, :, 0:2],
                    PT[mp],
                    start=(mp == 0),
                    stop=(mp == MP - 1),
                    perf_mode=DR,
                )
            # 1/colsum broadcast across partitions
            rinv = sm_pool.tile([1, 512], f32r, tag="rinv")
            with nc.allow_low_precision(reason="f32r softmax normalizer"):
                nc.vector.reciprocal(out=rinv, in_=cs_ps[0:1, :])
            ps_R = psum_o.tile([P, 512], f32, tag="ps_o", name="ps_R")
            nc.tensor.matmul(ps_R, ones_r, rinv, start=True, stop=True)
            Rsb = sm_pool.tile([P, 512], f32, tag="Rsb")
            nc.scalar.copy(out=Rsb, in_=ps_R)
            # PV into h2 fp8 c-pair tiles [128, 2, 512], normalizer folded in
            h28 = [
                h2_pool.tile([P, 2, 512], f8, tag=f"h2{t}", name="h2")
                for t in range(CP)
            ]
            for t2 in range(CP):
                for i2 in range(2):
                    ci = 2 * t2 + i2
                    ps_pv = psum_pv.tile([P, 512], f32, tag="ps_pv", name="ps_pv")
                    for mp in range(MP):
                        nc.tensor.matmul(
                            ps_pv,
                            v8[mp][:, :, ci * P : (ci + 1) * P],
                            PT[mp],
                            start=(mp == 0),
                            stop=(mp == MP - 1),
                            perf_mode=DR,
                        )
                    nc.vector.tensor_tensor(
                        out=h28[t2][:, i2, :], in0=ps_pv, in1=Rsb, op=OP.mult
                    )
            # output projection + bias + residual
            for oi in range(CT):
                ps_o = psum_o.tile([P, 512], f32, tag="ps_o", name="ps_o")
                for t2 in range(CP):
                    nc.tensor.matmul(
                        ps_o,
                        w8T["wp"][t2][:, :, oi * P : (oi + 1) * P],
                        h28[t2],
                        start=(t2 == 0),
                        stop=(t2 == CP - 1),
                        perf_mode=DR,
                    )
                ot = out_pool.tile([P, 512], f32, tag="out")
                nc.vector.scalar_tensor_tensor(
                    out=ot,
                    in0=ps_o,
                    scalar=bp_sb[:, oi : oi + 1],
                    in1=xg[oi][:, nq : nq + 512],
                    op0=OP.add,
                    op1=OP.add,
                )
                nc.gpsimd.dma_start(
                    out=out_d[oi * P : (oi + 1) * P, nq : nq + 512],
                    in_=ot,
                )
    body_scope.close()


_NC = {}


def _get_nc(reps=1):
    if reps not in _NC:
        _NC[reps] = build(reps)
    return _NC[reps]


def _consts():
    sel = np.zeros((P, GPT), np.float32)
    for rr in range(P):
        sel[rr, rr // GSZ] = 1.0
    selT = sel.T.copy()
    ident = np.eye(P, dtype=np.float32)
    return sel, selT, ident


def make_in_maps(inputs):
    x = np.ascontiguousarray(np.asarray(inputs["x"], dtype=np.float32))
    common = {}
    for nm in ("gn_scale", "gn_bias", "wq", "bq", "wk", "bk", "wv", "bv", "wp", "bp"):
        common[nm] = np.ascontiguousarray(np.asarray(inputs[nm], dtype=np.float32))
    # exact folds: softmax weights sum to 1, so v-bias passes through attention
    # unchanged and lands in the output projection as wp @ bv
    common["bp"] = np.ascontiguousarray(
        common["bp"] + common["wp"] @ common["bv"]
    ).astype(np.float32)
    sel, selT, ident = _consts()
    common["sel"] = sel
    common["selT"] = selT
    common["ident"] = ident
    return [dict(common, x=x[b]) for b in range(B)]


_EXEC = {}


def _get_exec(nc):
    """Build (once) the sharded jitted callable for the 8-core SPMD program."""
    key = id(nc)
    if key in _EXEC:
        return _EXEC[key]
    import jax
    from jax.sharding import Mesh, NamedSharding, PartitionSpec
    from jax.experimental.shard_map import shard_map
    from concourse.bass2jax import _bass_exec_p, install_neuronx_cc_hook

    install_neuronx_cc_hook()
    in_names, out_names, out_avals = [], [], []
    for alloc in nc.m.functions[0].allocations:
        if not isinstance(alloc, mybir.MemoryLocationSet):
            continue
        name = alloc.memorylocations[0].name
        if alloc.kind == "ExternalInput":
            in_names.append(name)
        elif alloc.kind == "ExternalOutput":
            out_names.append(name)
            out_avals.append(
                jax.core.ShapedArray(tuple(alloc.tensor_shape), mybir.dt.np(alloc.dtype))
            )
    all_names = in_names + out_names

    def _body(*args):
        return tuple(
            _bass_exec_p.bind(
                *args,
                out_avals=tuple(out_avals),
                in_names=tuple(all_names),
                out_names=tuple(out_names),
                lowering_input_output_aliases=(),
                sim_require_finite=True,
                sim_require_nnan=True,
                nc=nc,
            )
        )

    devices = jax.devices()[:B]
    mesh = Mesh(np.asarray(devices), ("core",))
    nsh = NamedSharding(mesh, PartitionSpec("core"))
    nsh_rep = NamedSharding(mesh, PartitionSpec())
    # x and partition_id differ per core; weights/consts are replicated so they
    # transfer once and fan out terminal-side.
    sharded_names = {"x", "partition_id"}
    in_specs = tuple(
        PartitionSpec("core") if nm in sharded_names else PartitionSpec()
        for nm in in_names
    ) + (PartitionSpec("core"),) * len(out_names)
    fn = jax.jit(
        shard_map(
            _body,
            mesh=mesh,
            in_specs=in_specs,
            out_specs=(PartitionSpec("core"),) * len(out_names),
            check_rep=False,
        ),
        keep_unused=True,
    )
    st = {
        "fn": fn,
        "in_names": in_names,
        "out_names": out_names,
        "out_avals": out_avals,
        "nsh": nsh,
        "nsh_rep": nsh_rep,
        "sharded_names": sharded_names,
        "hash": None,
        "dev_args": None,
    }
    _EXEC[key] = st
    return st


def kernel(_retried=False, **inputs):
    import hashlib

    import jax

    nc = _get_nc()
    st = _get_exec(nc)
    in_maps = make_in_maps(inputs)

    hsh = hashlib.md5()
    for nm in ("x", "gn_scale", "gn_bias", "wq", "bq", "wk", "bk", "wv", "bv", "wp", "bp"):
        hsh.update(np.ascontiguousarray(np.asarray(inputs[nm], np.float32)).tobytes())
    digest = hsh.digest()
    if st["hash"] != digest or st["dev_args"] is None:
        def _cv(c, nm):
            if nm in in_maps[c]:
                return np.asarray(in_maps[c][nm])
            for alloc in nc.m.functions[0].allocations:
                if (
                    isinstance(alloc, mybir.MemoryLocationSet)
                    and alloc.memorylocations[0].name == nm
                ):
                    return np.full(
                        tuple(alloc.tensor_shape), c, mybir.dt.np(alloc.dtype)
                    )
            raise KeyError(nm)

        dev_args = []
        for nm in st["in_names"]:
            if nm in st["sharded_names"]:
                a = np.concatenate([_cv(c, nm) for c in range(B)], axis=0)
                dev_args.append(jax.device_put(a, st["nsh"]))
            else:
                dev_args.append(jax.device_put(_cv(0, nm), st["nsh_rep"]))
        for a in st["out_avals"]:
            z = np.zeros((B * a.shape[0], *a.shape[1:]), a.dtype)
            dev_args.append(jax.device_put(z, st["nsh"]))
        st["dev_args"] = dev_args
        st["hash"] = digest

    try:
        r = st["fn"](*st["dev_args"])
        jax.block_until_ready(r)
    except Exception:
        # transient device error (e.g. NRT exec-unit wedge): re-place buffers
        # and retry once after a short backoff
        import time as _time

        _time.sleep(10.0)
        if _retried:
            raise
        st["hash"] = None
        st["dev_args"] = None
        return kernel(_retried=True, **inputs)
    out = np.asarray(r[0]).reshape(B, C, N)
    return out.astype(np.float32)
